# revision 20
# baseline (speedup 1.0000x reference)
"""TRN2 Bass kernel for OneLayerCNN: conv2d(4x4, stride 2, pad 2) + bias + ReLU.

Input  A_prev (64, 256, 256, 3) f32, W (4,4,3,16), b (1,1,1,16)
Output (64, 129*129*16) f32.

Data-parallel over 8 NeuronCores (8 images each). v3 design:

- Host pre-packs the input into matmul-ready fp16 "column strips": row PAIRS
  (2re, 2re+1) are column-interleaved (c = 2*(3x+ci) + rowparity), so one
  K<=121 band window spans TWO filter rows -> only 2 accumulating matmuls
  per output block (vs 4 with single-row banding), and the strips arrive
  transposed ([band-offset, (pair,img)]) so there are ZERO PE transposes
  and zero PSUM->SBUF transpose copies.
- The strips are shipped as 4 instance-GROUP arrays (group g = h-blocks
  2g, 2g+1, with the 16-instance seams duplicated) laid out so each group
  is ONE DMA with 8160B-contiguous per-partition descriptors, and each
  h-block's matmuls depend only on its own group's tile (fine-grained
  pipelining; PE starts after ~1MB, not after the full input).
- 15 w-blocks of 9 outputs (K=120+bias row, N=144), 9 h-blocks of 16 rows
  (M=128 = 16 h' x 8 img). Stationary operand = activation window (fp16 ->
  fast weight load), moving = banded weights.
- 3 w-blocks share one PSUM bank (3 x 144 cols = 1728B), evicted in one
  fused-ReLU op of [128, 432] (amortizes the per-op eviction overhead),
  alternating DVE/ACT.
- Bias rides a host-baked ones-row at partition K of each strip (tap-0
  matmul only). Zero pads are K-truncations of border blocks with
  host-shifted weight variants.
- fp16 everywhere off-chip (half the HBM traffic of f32); PSUM accumulates
  fp32; output is written fp16 and upcast to f32 on the host.
A post-pass splits multi-sem-wait instructions (walrus accepts one sync
wait per instruction). A short PE warmup during the initial DMA wait opens
the HAM clock gate before the real matmuls.
"""
import numpy as np
from contextlib import ExitStack

import concourse.bass as bass
import concourse.tile as tile
from concourse import mybir
from concourse.bass_utils import run_bass_kernel_spmd
import bass_rust

# ---------------- problem constants (hardcoded) ----------------
N_CORES = 8
IMG = 8              # images per core
H = 256
WID = 256
CIN = 3
F = 4
COUT = 16
HO = 129
WO = 129
RW = WID * CIN       # 768 floats per row
IC = 2 * RW          # 1536: row-pair interleaved width
S = 9                # w' outputs per w-block
NWB = 15             # w-blocks (14 full + 1 of 3 outputs)
NPAIR = 130          # row pairs incl. re=-1 and re=128 zero pads
NI = NPAIR * IMG     # 1040 instance columns total
NHB = 9              # h-blocks: 8 x 16 h' + 1 x 1 h'
OUTROW = WO * COUT   # 2064
NG = 4               # instance groups (h-block pairs; group 0 also holds b=8)
GW = 272             # instance columns per group (264 used + seam slack)
GW0 = 288            # group 0: + the 16 ragged b=8 instances (1024..1039)
GSTEP = 256          # group stride in global instance coords

DT = mybir.dt.float16
DT32 = mybir.dt.float32


def _bgeo(B):
    """w-block geometry: (c0 = window start in interleaved coords,
    K = band rows, N = matmul cols)."""
    c0 = max(0, 108 * B - 12)
    c1 = min(IC, 108 * B + 108)
    ns = min(S, WO - S * B)
    return c0, c1 - c0, ns * COUT


def _split_multi_waits(nc):
    """walrus accepts at most ONE sync wait per instruction; hoist extras
    onto NoOps inserted just before, same engine queue."""
    ctr = 0
    for f in nc.m.functions:
        for bb in f.blocks:
            insts = bb.instructions  # live list
            out = []
            changed = False
            for inst in insts:
                si = inst.sync_info
                if si is None:
                    out.append(inst)
                    continue
                waits = list(si.on_wait)
                if len(waits) > 1:
                    changed = True
                    for w in waits[:-1]:
                        ctr += 1
                        nop = mybir.InstNoOp(name=f"I-wsplit-{ctr}")
                        nop.engine = inst.engine
                        nop.sync_info = bass_rust.SyncInfo(
                            on_wait=[w], on_update=[])
                        out.append(nop)
                    inst.sync_info = bass_rust.SyncInfo(
                        on_wait=[waits[-1]], on_update=list(si.on_update))
                out.append(inst)
            if changed:
                insts[:] = out
    return nc


def _make_consts(W_arr, b_arr):
    """Banded weights for the pair-interleaved layout, fp16.

    wb[tap][12s + 2*(3fw+ci) + q, 16s+co] = W[2*tap+q, fw, ci, co]
    (tap = which row pair, q = row parity inside the pair). Variants:
      std  [121|120, 144]  rows 0..119 (+bias row 120 on tap0)
      B0   [109|108, 144]  rows 12..119 shifted to 0 (left pad dropped)
      B14  [ 37| 36,  48]  rows 0..35, 3 outputs (right pad dropped)
    Bias b[co] is baked into row K of each tap-0 variant (multiplied by the
    strips' ones-row). Packed into one [121, 672] tensor.
    """
    wb = np.zeros((2, 120, 144), dtype=np.float32)
    for tap in range(2):
        for s_ in range(S):
            for fw in range(F):
                for ci in range(CIN):
                    for q in range(2):
                        wb[tap, 12 * s_ + 2 * (3 * fw + ci) + q,
                           16 * s_:16 * s_ + 16] = W_arr[2 * tap + q, fw, ci]
    bias = b_arr.reshape(-1).astype(np.float32)
    # K is padded to 128 on device (FWL wants full-128 stationary operands);
    # rows >= the true K are ZERO here, which nullifies whatever sits in the
    # strip tiles' pad partitions.
    comb = np.zeros((128, 672), dtype=np.float16)
    comb[0:120, 0:144] = wb[0]
    comb[120, 0:144] = np.tile(bias, S)
    comb[0:120, 144:288] = wb[1]
    comb[0:108, 288:432] = wb[0][12:120]
    comb[108, 288:432] = np.tile(bias, S)
    comb[0:108, 432:576] = wb[1][12:120]
    comb[0:36, 576:624] = wb[0][0:36, 0:48]
    comb[36, 576:624] = np.tile(bias, 3)
    comb[0:36, 624:672] = wb[1][0:36, 0:48]
    return comb


def _make_strips(A_core):
    """Per-core input -> [121, 4*15*272] fp16 group-major strip tensor.

    G[img, re', c]: re' = re+1 (pairs -1..128), c = 2*flat + rowparity.
    Strip B = G[:, :, c0:c0+K] transposed to [K, (re', img)], ones row at K.
    Group g holds instance columns [256g, 256g+272) of every strip,
    contiguous per partition: out[p, 4080*g + 272*B + i] .
    """
    A16 = A_core.reshape(IMG, H, RW).astype(np.float16)
    G = np.zeros((IMG, NPAIR, IC), dtype=np.float16)
    G[:, 1:129, 0::2] = A16[:, 0::2, :]
    G[:, 1:129, 1::2] = A16[:, 1::2, :]
    full = np.zeros((121, NWB, NI), dtype=np.float16)
    for B in range(NWB):
        c0, K, _ = _bgeo(B)
        full[0:K, B] = np.transpose(G[:, :, c0:c0 + K], (2, 1, 0)
                                    ).reshape(K, NI)
        full[K, B] = 1.0
    parts = []
    g0 = np.concatenate([full[:, :, 0:GW],
                         full[:, :, NI - 16:NI]], axis=2)   # + b=8 insts
    parts.append(g0.reshape(121, NWB * GW0))
    for g in range(1, NG):
        parts.append(np.ascontiguousarray(
            full[:, :, GSTEP * g:GSTEP * g + GW]).reshape(121, NWB * GW))
    return np.ascontiguousarray(np.concatenate(parts, axis=1))


def _build_nc():
    nc = bass.Bass()
    NCOL = NWB * (GW0 + (NG - 1) * GW)
    a_in = nc.declare_dram_parameter("A", [121, NCOL], DT, isOutput=False)
    c_in = nc.declare_dram_parameter("consts", [128, 672], DT, isOutput=False)
    z_out = nc.declare_dram_parameter("Z", [IMG, HO, OUTROW], DT,
                                      isOutput=True)

    with tile.TileContext(nc) as tc, ExitStack() as ctx:
        consts = ctx.enter_context(tc.tile_pool(name="consts", bufs=1))
        spool = ctx.enter_context(tc.tile_pool(name="strips", bufs=1))
        opool = ctx.enter_context(tc.tile_pool(name="oacc", bufs=3))
        ppool = ctx.enter_context(
            tc.tile_pool(name="pconv", bufs=7, space="PSUM"))
        pw_pool = ctx.enter_context(
            tc.tile_pool(name="pwarm", bufs=1, space="PSUM"))

        # input strips first: 3 sub-tiles per group (5 strips each), each its
        # own tile so matmuls gate on exactly the 0.35MB they read. Only
        # partitions 0..120 come from HBM; 121..127 (the FWL pad) are zeroed
        # by a 32-aligned memset that the DMA then partially overwrites (WAW
        # order is program order).
        sgt = []       # sgt[g][s3] tile, strip B at cols (B%5)*GWg
        off = 0
        for g in range(NG):
            GWg = GW0 if g == 0 else GW
            row = []
            for s3 in range(3):
                t = spool.tile([128, 5 * GWg], DT, tag=f"sg{g}_{s3}",
                               name=f"sg{g}_{s3}")
                nc.gpsimd.memset(t[96:128, :], 0.0)
                nc.sync.dma_start(out=t[0:121, :],
                                  in_=a_in[:, off:off + 5 * GWg])
                off += 5 * GWg
                row.append(t)
            sgt.append(row)

        call = consts.tile([128, 672], DT, tag="call", name="call")
        nc.scalar.dma_start(out=call[:], in_=c_in[:])
        # K padded to 128: weight pad rows are zero, so whatever sits in the
        # strip pad partitions contributes nothing
        wstd = (call[:, 0:144], call[:, 144:288])
        wb0 = (call[:, 288:432], call[:, 432:576])
        wb14 = (call[:, 576:624], call[:, 624:672])

        # PE warmup: dummy matmuls during the initial DMA wait so the HAM
        # clock gate opens (1.2 -> 2.4 GHz) before the real work starts
        wtile = consts.tile([128, 640], DT, tag="wtile", name="wtile")
        nc.gpsimd.memset(wtile[:], 0.0)
        pwarm = pw_pool.tile([128, 512], DT32, tag="pwarm", name="pwarm")
        for _ in range(8):
            nc.tensor.matmul(pwarm[:], wtile[0:128, 0:128],
                             wtile[0:128, 128:640], start=True, stop=True)

        ev = 0
        # ragged h-block (b=8, h'=128) first: keeps it off the tail; its
        # instances are duplicated at cols [272:288) of group 0
        for b in [8] + list(range(8)):
            g = 0 if b == 8 else b // 2
            GWg = GW0 if g == 0 else GW
            il = 272 if b == 8 else 128 * b - GSTEP * g   # 0 | 128
            nh = 16 if b < 8 else 1
            m = 8 * nh
            oacc = opool.tile([128, OUTROW], DT, tag="oacc")
            for E in range(5):           # 5 PSUM banks x 3 w-blocks
                pc = ppool.tile([128, 432], DT32, tag="pc")
                for j in range(3):
                    B = 3 * E + j
                    _, _, N = _bgeo(B)
                    w0, w1 = wb0 if B == 0 else (
                        wb14 if B == NWB - 1 else wstd)
                    st = sgt[g][B // 5]
                    i0 = GWg * (B % 5) + il
                    nc.tensor.matmul(
                        pc[0:m, 144 * j:144 * j + N],
                        st[0:128, i0:i0 + m], w0[0:128, 0:N],
                        start=True, stop=False)
                    nc.tensor.matmul(
                        pc[0:m, 144 * j:144 * j + N],
                        st[0:128, i0 + 8:i0 + 8 + m], w1[0:128, 0:N],
                        start=False, stop=True)
                nE = 432 if E < 4 else 336
                dst = oacc[0:m, 432 * E:432 * E + nE]
                # fused ReLU eviction; alternate DVE/ACT by global parity
                if ev % 2 == 1:
                    nc.scalar.activation(dst, pc[0:m, 0:nE],
                                         mybir.ActivationFunctionType.Relu)
                else:
                    nc.vector.tensor_scalar_max(dst, pc[0:m, 0:nE], 0.0)
                ev += 1
            h0 = 16 * b
            # output DMAs ride the sync queue (the scalar queue's sequencer
            # runs the ACT evictions and must not stall on DGE work), split
            # in two so bytes start draining after the E2 eviction
            zr = z_out[:, h0:h0 + nh, :].rearrange("i j c -> j i c")
            nc.sync.dma_start(out=zr[:, :, 0:1296], in_=oacc[0:m, 0:1296])
            nc.sync.dma_start(out=zr[:, :, 1296:OUTROW],
                              in_=oacc[0:m, 1296:OUTROW])

    _split_multi_waits(nc)
    return nc


_NC_CACHE = {}


def _get_nc():
    if "nc" not in _NC_CACHE:
        _NC_CACHE["nc"] = _build_nc()
    return _NC_CACHE["nc"]


def kernel(A_prev, W, b, _trace=False, _dt=None):
    A_prev = np.ascontiguousarray(A_prev, dtype=np.float32)
    W = np.asarray(W, dtype=np.float32)
    b = np.asarray(b, dtype=np.float32)
    comb = _make_consts(W, b)

    nc = _get_nc()
    in_maps = []
    for c in range(N_CORES):
        strips = _make_strips(A_prev[c * IMG:(c + 1) * IMG])
        in_maps.append({"A": strips, "consts": comb})

    res = run_bass_kernel_spmd(nc, in_maps, list(range(N_CORES)),
                               trace=_trace)
    out = np.concatenate(
        [res.results[c]["Z"].astype(np.float32).reshape(IMG, -1)
         for c in range(N_CORES)], axis=0)
    if _trace:
        return out, res
    return out


# revision 21
# speedup vs baseline: 1.2233x; 1.2233x over previous
"""TRN2 Bass kernel for OneLayerCNN: conv2d(4x4, stride 2, pad 2) + bias + ReLU.

Input  A_prev (64, 256, 256, 3) f32, W (4,4,3,16), b (1,1,1,16)
Output (64, 129*129*16) f32.

Data-parallel over 8 NeuronCores (8 images each). v3 design:

- Host pre-packs the input into matmul-ready fp16 "column strips": row PAIRS
  (2re, 2re+1) are column-interleaved (c = 2*(3x+ci) + rowparity), so one
  K<=121 band window spans TWO filter rows -> only 2 accumulating matmuls
  per output block (vs 4 with single-row banding), and the strips arrive
  transposed ([band-offset, (pair,img)]) so there are ZERO PE transposes
  and zero PSUM->SBUF transpose copies.
- The strips are shipped as 4 instance-GROUP arrays (group g = h-blocks
  2g, 2g+1, with the 16-instance seams duplicated) laid out so each group
  is ONE DMA with 8160B-contiguous per-partition descriptors, and each
  h-block's matmuls depend only on its own group's tile (fine-grained
  pipelining; PE starts after ~1MB, not after the full input).
- 15 w-blocks of 9 outputs (K=120+bias row, N=144), 9 h-blocks of 16 rows
  (M=128 = 16 h' x 8 img). Stationary operand = activation window (fp16 ->
  fast weight load), moving = banded weights.
- 3 w-blocks share one PSUM bank (3 x 144 cols = 1728B), evicted in one
  fused-ReLU op of [128, 432] (amortizes the per-op eviction overhead),
  alternating DVE/ACT.
- Bias rides a host-baked ones-row at partition K of each strip (tap-0
  matmul only). Zero pads are K-truncations of border blocks with
  host-shifted weight variants.
- fp16 everywhere off-chip (half the HBM traffic of f32); PSUM accumulates
  fp32; output is written fp16 and upcast to f32 on the host.
A post-pass splits multi-sem-wait instructions (walrus accepts one sync
wait per instruction). A short PE warmup during the initial DMA wait opens
the HAM clock gate before the real matmuls.
"""
import numpy as np
from contextlib import ExitStack

import concourse.bass as bass
import concourse.tile as tile
from concourse import mybir
from concourse.bass_utils import run_bass_kernel_spmd
import bass_rust

# ---------------- problem constants (hardcoded) ----------------
N_CORES = 8
IMG = 8              # images per core
H = 256
WID = 256
CIN = 3
F = 4
COUT = 16
HO = 129
WO = 129
RW = WID * CIN       # 768 floats per row
IC = 2 * RW          # 1536: row-pair interleaved width
S = 9                # w' outputs per w-block
NWB = 15             # w-blocks (14 full + 1 of 3 outputs)
NPAIR = 130          # row pairs incl. re=-1 and re=128 zero pads
NI = NPAIR * IMG     # 1040 instance columns total
NHB = 9              # h-blocks: 8 x 16 h' + 1 x 1 h'
OUTROW = WO * COUT   # 2064
NG = 4               # instance groups (h-block pairs; group 0 also holds b=8)
GW = 272             # instance columns per group (264 used + seam slack)
GW0 = 288            # group 0: + the 16 ragged b=8 instances (1024..1039)
GSTEP = 256          # group stride in global instance coords

DT = mybir.dt.float16
DT32 = mybir.dt.float32


def _bgeo(B):
    """w-block geometry: (c0 = window start in interleaved coords,
    K = band rows, N = matmul cols)."""
    c0 = max(0, 108 * B - 12)
    c1 = min(IC, 108 * B + 108)
    ns = min(S, WO - S * B)
    return c0, c1 - c0, ns * COUT


def _split_multi_waits(nc):
    """walrus accepts at most ONE sync wait per instruction; hoist extras
    onto NoOps inserted just before, same engine queue."""
    ctr = 0
    for f in nc.m.functions:
        for bb in f.blocks:
            insts = bb.instructions  # live list
            out = []
            changed = False
            for inst in insts:
                si = inst.sync_info
                if si is None:
                    out.append(inst)
                    continue
                waits = list(si.on_wait)
                if len(waits) > 1:
                    changed = True
                    for w in waits[:-1]:
                        ctr += 1
                        nop = mybir.InstNoOp(name=f"I-wsplit-{ctr}")
                        nop.engine = inst.engine
                        nop.sync_info = bass_rust.SyncInfo(
                            on_wait=[w], on_update=[])
                        out.append(nop)
                    inst.sync_info = bass_rust.SyncInfo(
                        on_wait=[waits[-1]], on_update=list(si.on_update))
                out.append(inst)
            if changed:
                insts[:] = out
    return nc


def _make_consts(W_arr, b_arr):
    """Banded weights for the pair-interleaved layout, fp16.

    wb[tap][12s + 2*(3fw+ci) + q, 16s+co] = W[2*tap+q, fw, ci, co]
    (tap = which row pair, q = row parity inside the pair). Variants:
      std  [121|120, 144]  rows 0..119 (+bias row 120 on tap0)
      B0   [109|108, 144]  rows 12..119 shifted to 0 (left pad dropped)
      B14  [ 37| 36,  48]  rows 0..35, 3 outputs (right pad dropped)
    Bias b[co] is baked into row K of each tap-0 variant (multiplied by the
    strips' ones-row). Packed into one [121, 672] tensor.
    """
    wb = np.zeros((2, 120, 144), dtype=np.float32)
    for tap in range(2):
        for s_ in range(S):
            for fw in range(F):
                for ci in range(CIN):
                    for q in range(2):
                        wb[tap, 12 * s_ + 2 * (3 * fw + ci) + q,
                           16 * s_:16 * s_ + 16] = W_arr[2 * tap + q, fw, ci]
    bias = b_arr.reshape(-1).astype(np.float32)
    # K is padded to 128 on device (FWL wants full-128 stationary operands);
    # rows >= the true K are ZERO here, which nullifies whatever sits in the
    # strip tiles' pad partitions.
    comb = np.zeros((128, 672), dtype=np.float16)
    comb[0:120, 0:144] = wb[0]
    comb[120, 0:144] = np.tile(bias, S)
    comb[0:120, 144:288] = wb[1]
    comb[0:108, 288:432] = wb[0][12:120]
    comb[108, 288:432] = np.tile(bias, S)
    comb[0:108, 432:576] = wb[1][12:120]
    comb[0:36, 576:624] = wb[0][0:36, 0:48]
    comb[36, 576:624] = np.tile(bias, 3)
    comb[0:36, 624:672] = wb[1][0:36, 0:48]
    return comb


def _make_strips(A_core):
    """Per-core input -> [121, 4*15*272] fp16 group-major strip tensor.

    G[img, re', c]: re' = re+1 (pairs -1..128), c = 2*flat + rowparity.
    Strip B = G[:, :, c0:c0+K] transposed to [K, (re', img)], ones row at K.
    Group g holds instance columns [256g, 256g+272) of every strip,
    contiguous per partition: out[p, 4080*g + 272*B + i] .
    """
    A16 = A_core.reshape(IMG, H, RW).astype(np.float16)
    G = np.zeros((IMG, NPAIR, IC), dtype=np.float16)
    G[:, 1:129, 0::2] = A16[:, 0::2, :]
    G[:, 1:129, 1::2] = A16[:, 1::2, :]
    full = np.zeros((128, NWB, NI), dtype=np.float16)
    for B in range(NWB):
        c0, K, _ = _bgeo(B)
        full[0:K, B] = np.transpose(G[:, :, c0:c0 + K], (2, 1, 0)
                                    ).reshape(K, NI)
        full[K, B] = 1.0
    parts = []
    g0 = np.concatenate([full[:, :, 0:GW],
                         full[:, :, NI - 16:NI]], axis=2)   # + b=8 insts
    parts.append(g0.reshape(128, NWB * GW0))
    for g in range(1, NG):
        parts.append(np.ascontiguousarray(
            full[:, :, GSTEP * g:GSTEP * g + GW]).reshape(128, NWB * GW))
    return np.ascontiguousarray(np.concatenate(parts, axis=1))


def _build_nc():
    nc = bass.Bass()
    NCOL = NWB * (GW0 + (NG - 1) * GW)
    a_in = nc.declare_dram_parameter("A", [128, NCOL], DT, isOutput=False)
    c_in = nc.declare_dram_parameter("consts", [128, 672], DT, isOutput=False)
    z_out = nc.declare_dram_parameter("Z", [IMG, HO, OUTROW], DT,
                                      isOutput=True)

    with tile.TileContext(nc) as tc, ExitStack() as ctx:
        consts = ctx.enter_context(tc.tile_pool(name="consts", bufs=1))
        spool = ctx.enter_context(tc.tile_pool(name="strips", bufs=1))
        opool = ctx.enter_context(tc.tile_pool(name="oacc", bufs=3))
        ppool = ctx.enter_context(
            tc.tile_pool(name="pconv", bufs=7, space="PSUM"))
        pw_pool = ctx.enter_context(
            tc.tile_pool(name="pwarm", bufs=1, space="PSUM"))

        # input strips first: 3 sub-tiles per group (5 strips each), each its
        # own tile so matmuls gate on exactly the 0.35MB they read
        sgt = []       # sgt[g][s3] tile, strip B at cols (B%5)*GWg
        off = 0
        for g in range(NG):
            GWg = GW0 if g == 0 else GW
            row = []
            for s3 in range(3):
                t = spool.tile([128, 5 * GWg], DT, tag=f"sg{g}_{s3}",
                               name=f"sg{g}_{s3}")
                nc.sync.dma_start(out=t[:], in_=a_in[:, off:off + 5 * GWg])
                off += 5 * GWg
                row.append(t)
            sgt.append(row)

        call = consts.tile([128, 672], DT, tag="call", name="call")
        nc.scalar.dma_start(out=call[:], in_=c_in[:])
        # K padded to 128: weight pad rows are zero, so whatever sits in the
        # strip pad partitions contributes nothing
        wstd = (call[:, 0:144], call[:, 144:288])
        wb0 = (call[:, 288:432], call[:, 432:576])
        wb14 = (call[:, 576:624], call[:, 624:672])

        # PE warmup: dummy matmuls during the initial DMA wait so the HAM
        # clock gate opens (1.2 -> 2.4 GHz) before the real work starts
        wtile = consts.tile([128, 640], DT, tag="wtile", name="wtile")
        nc.gpsimd.memset(wtile[:], 0.0)
        pwarm = pw_pool.tile([128, 512], DT32, tag="pwarm", name="pwarm")
        for _ in range(8):
            nc.tensor.matmul(pwarm[:], wtile[0:128, 0:128],
                             wtile[0:128, 128:640], start=True, stop=True)

        ev = 0
        # ragged h-block (b=8, h'=128) first: keeps it off the tail; its
        # instances are duplicated at cols [272:288) of group 0
        for b in [8] + list(range(8)):
            g = 0 if b == 8 else b // 2
            GWg = GW0 if g == 0 else GW
            il = 272 if b == 8 else 128 * b - GSTEP * g   # 0 | 128
            nh = 16 if b < 8 else 1
            m = 8 * nh
            oacc = opool.tile([128, OUTROW], DT, tag="oacc")
            for E in range(5):           # 5 PSUM banks x 3 w-blocks
                pc = ppool.tile([128, 432], DT32, tag="pc")
                for j in range(3):
                    B = 3 * E + j
                    _, _, N = _bgeo(B)
                    w0, w1 = wb0 if B == 0 else (
                        wb14 if B == NWB - 1 else wstd)
                    st = sgt[g][B // 5]
                    i0 = GWg * (B % 5) + il
                    nc.tensor.matmul(
                        pc[0:m, 144 * j:144 * j + N],
                        st[0:128, i0:i0 + m], w0[0:128, 0:N],
                        start=True, stop=False)
                    nc.tensor.matmul(
                        pc[0:m, 144 * j:144 * j + N],
                        st[0:128, i0 + 8:i0 + 8 + m], w1[0:128, 0:N],
                        start=False, stop=True)
                nE = 432 if E < 4 else 336
                dst = oacc[0:m, 432 * E:432 * E + nE]
                # fused ReLU eviction; alternate DVE/ACT by global parity
                if ev % 2 == 1:
                    nc.scalar.activation(dst, pc[0:m, 0:nE],
                                         mybir.ActivationFunctionType.Relu)
                else:
                    nc.vector.tensor_scalar_max(dst, pc[0:m, 0:nE], 0.0)
                ev += 1
            h0 = 16 * b
            # output DMAs ride the sync queue (the scalar queue's sequencer
            # runs the ACT evictions and must not stall on DGE work), split
            # in two so bytes start draining after the E2 eviction
            zr = z_out[:, h0:h0 + nh, :].rearrange("i j c -> j i c")
            nc.sync.dma_start(out=zr[:, :, 0:1296], in_=oacc[0:m, 0:1296])
            nc.sync.dma_start(out=zr[:, :, 1296:OUTROW],
                              in_=oacc[0:m, 1296:OUTROW])

    _split_multi_waits(nc)
    return nc


_NC_CACHE = {}


def _get_nc():
    if "nc" not in _NC_CACHE:
        _NC_CACHE["nc"] = _build_nc()
    return _NC_CACHE["nc"]


def kernel(A_prev, W, b, _trace=False, _dt=None):
    A_prev = np.ascontiguousarray(A_prev, dtype=np.float32)
    W = np.asarray(W, dtype=np.float32)
    b = np.asarray(b, dtype=np.float32)
    comb = _make_consts(W, b)

    nc = _get_nc()
    in_maps = []
    for c in range(N_CORES):
        strips = _make_strips(A_prev[c * IMG:(c + 1) * IMG])
        in_maps.append({"A": strips, "consts": comb})

    res = run_bass_kernel_spmd(nc, in_maps, list(range(N_CORES)),
                               trace=_trace)
    out = np.concatenate(
        [res.results[c]["Z"].astype(np.float32).reshape(IMG, -1)
         for c in range(N_CORES)], axis=0)
    if _trace:
        return out, res
    return out
